# revision 51
# baseline (speedup 1.0000x reference)
"""Multi-head self-attention on 8 TRN2 NeuronCores, batch-data-parallel.

Problem (hardcoded): inputs (8, 1024, 1024) f32, Wq/Wk/Wv (1024, 1024) f32,
heads=16, head_dim=64. out[b,q,h,v] = softmax(Q K^T / 8) V per head.

Sharding: batch b -> core b (8 cores, one batch element each, weights
replicated). No collectives needed.

Per-core dataflow (all matmuls bf16, PSUM fp32):
  xT = transpose(x)                     (PE transpose, d on partitions)
  QT[p] = Wq[:,chunk].T @ xT            ((head*kdim) on partitions)
  KT[p] likewise; V = xT.T @ Wv         (m on partitions, natural)
  scoresT[m,q] = KT.T @ QT per head     (two heads row-packed, K=64 each)
  attnT = exp(scoresT / 8)              (ScalarE, PSUM -> SBUF bf16)
  outT[v,q] (+ sums row) = [V|1].T @ attnT
  out = transpose(outT) / sums          (PE transpose + DVE normalize)

The main loop is a software pipeline over head pairs: in "phase" p the PE
interleaves, per m-chunk step s: scores(p) matmuls, outT(p-1) accumulation,
QT/KT(p+1) projection, and out-transposes of older pairs -- so the PE never
idles long enough for the HAM clock gate to re-throttle, and ScalarE's exp
stream (the second-busiest engine) runs continuously.
"""

import numpy as np

import concourse.bass as bass
import concourse.mybir as mybir
from concourse import bacc
from concourse.tile import TileContext
from concourse.bass_utils import run_bass_kernel_spmd
from contextlib import ExitStack

F32 = mybir.dt.float32
BF16 = mybir.dt.bfloat16

B, W, D = 8, 1024, 1024
H, DK = 16, 64
P = 128
NT = W // P        # 8 partition tiles along q / d / m
NPAIR = H // 2     # 8 head pairs; pair p = heads (2p, 2p+1)
SCALE = float(DK) ** -0.5


def build_nc():
    import time as _time

    _tb = _time.time()
    print("[kernel] building bass graph...", flush=True)
    nc = bacc.Bacc("TRN2", target_bir_lowering=False, debug=False, num_devices=B)
    # Inputs arrive pre-cast to bf16 and pre-arranged on the host:
    # xt[pp, dt, q] = x[q, 128*dt+pp]; w*[pp, dt, n] = W[128*dt+pp, n].
    xt_d = nc.dram_tensor("xt", [P, NT, W], BF16, kind="ExternalInput").ap()
    # wq/wk are pair-major: [pp, pair, d_tile, col] so each projection lhsT
    # block and the priority DMA slice for pair 0 are contiguous.
    wq_d = nc.dram_tensor("wq", [P, NPAIR, NT, P], BF16, kind="ExternalInput").ap()
    wk_d = nc.dram_tensor("wk", [P, NPAIR, NT, P], BF16, kind="ExternalInput").ap()
    wv_d = nc.dram_tensor("wv", [P, NT, H * DK], BF16, kind="ExternalInput").ap()
    out_d = nc.dram_tensor("out", [W, H * DK], F32, kind="ExternalOutput").ap()

    with TileContext(nc) as tc, ExitStack() as ctx:
        big = ctx.enter_context(tc.tile_pool(name="big", bufs=1))
        xT = big.tile([P, NT, W], BF16)           # [d_in_tile, d_tile, q]
        wq_sb = big.tile([P, NPAIR, NT, P], BF16)  # [d_in_tile, pair, d_tile, col]
        wk_sb = big.tile([P, NPAIR, NT, P], BF16)
        wv_sb = big.tile([P, NT, H * DK], BF16)   # [d_in_tile, d_tile, hv]
        vo = big.tile([P, NT, H, DK + 1], BF16)   # [m_in_tile, m_tile, head, v|1]

        # PSUM budget (8 banks): psS 3x(128,1024)f32 = 6 (scores pipeline depth
        # 3; V chains borrow the third slot during phase 0), psP 1x(128,512)f32
        # = 1, psO 1x(65,512)f32 = 1.
        psS = ctx.enter_context(tc.tile_pool(name="psS", bufs=3, space="PSUM"))
        psP = ctx.enter_context(tc.tile_pool(name="psP", bufs=1, space="PSUM"))
        psO = ctx.enter_context(tc.tile_pool(name="psO", bufs=1, space="PSUM"))

        qk_pool = ctx.enter_context(tc.tile_pool(name="qk", bufs=2))
        attn_pool = ctx.enter_context(tc.tile_pool(name="attn", bufs=2))
        ot_pool = ctx.enter_context(tc.tile_pool(name="ot", bufs=4))
        ott_pool = ctx.enter_context(tc.tile_pool(name="ott", bufs=4))
        outp_pool = ctx.enter_context(tc.tile_pool(name="outp", bufs=2))
        small_pool = ctx.enter_context(tc.tile_pool(name="small", bufs=4))

        out_view = out_d.rearrange("(t r) n -> r t n", r=P)

        qt_tiles, kt_tiles, attn_tiles, ot_tiles, outp_tiles = {}, {}, {}, {}, {}
        ott_tiles = {}

        proj_state = {}

        def emit_proj_steps(pp, s):
            """Projection work for next pair pp at step s. Each (tensor, nh)
            chain of 8 matmuls is split 4+4 across two steps so score matmuls
            (which feed ScalarE) are never queued behind a full chain."""
            if pp > NPAIR - 1:
                return
            if s == 0:
                qt_tiles[pp] = qk_pool.tile([P, W], BF16, tag="qt", name=f"qt{pp}")
                kt_tiles[pp] = qk_pool.tile([P, W], BF16, tag="kt", name=f"kt{pp}")
            which = s // 2  # 0: QTnh0, 1: KTnh0, 2: QTnh1, 3: KTnh1
            w_sb, dst = [
                (wq_sb, qt_tiles[pp]),
                (wk_sb, kt_tiles[pp]),
                (wq_sb, qt_tiles[pp]),
                (wk_sb, kt_tiles[pp]),
            ][which]
            nh = which // 2
            if s % 2 == 0:
                # The prologue (pair 0) runs before any scores exist, so its
                # chains use the idle psS slots to avoid single-slot
                # serialization; in-loop chains use psP.
                pool, tag = (psS, "psS") if pp == 0 else (psP, "psP")
                ps = pool.tile([P, 512], F32, tag=tag, name=f"pp{pp}_{which}")
                proj_state["ps"] = ps
                kds = range(0, 4)
            else:
                ps = proj_state["ps"]
                kds = range(4, NT)
            for kd in kds:
                nc.tensor.matmul(
                    ps[:],
                    lhsT=w_sb[:, pp, kd, :],
                    rhs=xT[:, kd, 512 * nh : 512 * (nh + 1)],
                    start=(kd == 0),
                    stop=(kd == NT - 1),
                )
            if s % 2 == 1:
                nc.vector.tensor_copy(
                    out=dst[:, 512 * nh : 512 * (nh + 1)], in_=ps[:]
                )

        def emit_scores_step(p, s):
            qt_t, kt_t, attnT = qt_tiles[p], kt_tiles[p], attn_tiles[p]
            for hh in range(2):
                ps = psS.tile([P, W], F32, tag="psS")
                lo, hi = DK * hh, DK * (hh + 1)
                for nh in range(2):
                    nc.tensor.matmul(
                        ps[:, 512 * nh : 512 * (nh + 1)],
                        lhsT=kt_t[lo:hi, P * s : P * (s + 1)],
                        rhs=qt_t[lo:hi, 512 * nh : 512 * (nh + 1)],
                        start=True,
                        stop=True,
                        tile_position=(DK * hh, 0),
                    )
                nc.scalar.activation(
                    attnT[:, s, hh, :],
                    ps[:],
                    mybir.ActivationFunctionType.Exp,
                    scale=SCALE,
                )

        out_state = {}

        def emit_out_step(p, s):
            """outT accumulation for pair p. One (head, nh) chain of 8 chunk
            matmuls spans two steps through a single 1-bank psO slot:
            (h0,nh0) s0-1, (h0,nh1) s2-3, (h1,nh0) s4-5, (h1,nh1) s6-7."""
            attnT = attn_tiles[p]
            k = s // 2
            hh, nh = k // 2, k % 2
            h = 2 * p + hh
            if s % 2 == 0:
                out_state["ps"] = psO.tile(
                    [DK + 1, 512], F32, tag="psO", name=f"psO{p}_{s}"
                )
            ps_o = out_state["ps"]
            for c in range(4 * (s % 2), 4 * (s % 2) + 4):
                nc.tensor.matmul(
                    ps_o[:],
                    lhsT=vo[:, c, h, :],
                    rhs=attnT[:, c, hh, 512 * nh : 512 * (nh + 1)],
                    start=(c == 0),
                    stop=(c == NT - 1),
                )
            if s % 2 == 1:
                if nh == 0:
                    # 80 partitions so the xbar transpose DMA constraint
                    # (mult of 16) holds; rows 65-79 are never read back.
                    ot_tiles[(p, hh)] = ot_pool.tile(
                        [80, W], BF16, tag="ot", name=f"ot{p}_{hh}"
                    )
                oT = ot_tiles[(p, hh)]
                nc.vector.tensor_copy(
                    out=oT[0 : DK + 1, 512 * nh : 512 * (nh + 1)], in_=ps_o[:]
                )
                if nh == 1:
                    oTT = ott_pool.tile(
                        [P, NT, 80], BF16, tag="ott", name=f"ott{p}_{hh}"
                    )
                    nc.sync.dma_start_transpose(oTT[:], oT[:])
                    ott_tiles[(p, hh)] = oTT

        def emit_out_last(p, hh):
            """Epilogue outT for the final pair: both nh chains at once through
            psO and psP (psP is free -- there is no proj(8))."""
            attnT = attn_tiles[p]
            h = 2 * p + hh
            oT = ot_pool.tile([80, W], BF16, tag="ot", name=f"ot{p}_{hh}")
            ot_tiles[(p, hh)] = oT
            for nh in range(2):
                pool, tag = (psO, "psO") if nh == 0 else (psP, "psP")
                ps_o = pool.tile(
                    [DK + 1, 512], F32, tag=tag, name=f"last{p}_{hh}_{nh}"
                )
                for c in range(NT):
                    nc.tensor.matmul(
                        ps_o[:],
                        lhsT=vo[:, c, h, :],
                        rhs=attnT[:, c, hh, 512 * nh : 512 * (nh + 1)],
                        start=(c == 0),
                        stop=(c == NT - 1),
                    )
                nc.vector.tensor_copy(
                    out=oT[0 : DK + 1, 512 * nh : 512 * (nh + 1)], in_=ps_o[:]
                )
            oTT = ott_pool.tile([P, NT, 80], BF16, tag="ott", name=f"ott{p}_{hh}")
            nc.sync.dma_start_transpose(oTT[:], oT[:])
            ott_tiles[(p, hh)] = oTT

        rec_tiles = {}

        def emit_norm_step(p, hh, s):
            """Normalize two q-chunks (2s', 2s'+1) of the transposed output
            oTT[(p, hh)] into out_pair(p): per-partition reciprocal multiply."""
            key = (p, hh)
            if key not in ott_tiles:
                return
            oTT = ott_tiles[key]
            out_pair = outp_tiles[p]
            sp = s % 4
            if sp == 0:
                rec = small_pool.tile([P, NT], F32, tag="rec", name=f"rec{p}_{hh}")
                nc.vector.reciprocal(rec[:], oTT[:, :, DK])
                rec_tiles[key] = rec
            rec = rec_tiles[key]
            for t in (2 * sp, 2 * sp + 1):
                nc.vector.tensor_scalar_mul(
                    out_pair[:, t, DK * hh : DK * (hh + 1)],
                    oTT[:, t, 0:DK],
                    rec[:, t : t + 1],
                )

        def emit_out_dma(p):
            nc.sync.dma_start(
                out=out_view[:, :, P * p : P * (p + 1)], in_=outp_tiles[p][:]
            )

        # ---- prologue: load inputs (already bf16 + transposed on host).
        # All DMAs share the 16 SDMA engines, so prioritize by first use:
        # pair-0 weight column slices (tiny) land almost immediately, letting
        # the first projection chains start as soon as xT arrives; wv (first
        # used mid-phase-0) goes last. sync and scalar are separate HWDGE
        # FIFOs, so the two streams interleave.
        # xt + the two tiny pair-0 weight slices unblock the whole prologue;
        # wv next (first V chain runs at phase-0 step 1), bulk weights last.
        nc.scalar.dma_start(out=wq_sb[:, 0], in_=wq_d[:, 0])
        nc.scalar.dma_start(out=wk_sb[:, 0], in_=wk_d[:, 0])
        nc.sync.dma_start(out=xT[:], in_=xt_d[:])
        nc.scalar.dma_start(out=wq_sb[:, 1:], in_=wq_d[:, 1:])
        nc.sync.dma_start(out=wv_sb[:], in_=wv_d[:])
        nc.sync.dma_start(out=wk_sb[:, 1:], in_=wk_d[:, 1:])

        def emit_v_step(j):
            """V projection for m-tile j (16 MMs) through the third psS slot
            (scores only keep 2 tiles in flight during phase 0)."""
            ps = psS.tile([P, W], F32, tag="psS", name=f"vchain{j}")
            for kd in range(NT):
                first, last = kd == 0, kd == NT - 1
                for nh in range(2):
                    nc.tensor.matmul(
                        ps[:, 512 * nh : 512 * (nh + 1)],
                        lhsT=xT[:, kd, P * j : P * (j + 1)],
                        rhs=wv_sb[:, kd, 512 * nh : 512 * (nh + 1)],
                        start=first,
                        stop=last,
                    )
            nc.vector.tensor_copy(
                out=vo[:, j, :, 0:DK],
                in_=ps.rearrange("p (h v) -> p h v", v=DK),
            )
            nc.vector.memset(vo[:, j, :, DK : DK + 1], 1.0)

        # QT/KT for pair 0.
        for s in range(8):
            emit_proj_steps(0, s)

        # ---- main pipeline over phases ----
        for ph in range(NPAIR + 1):
            if ph < NPAIR:
                attn_tiles[ph] = attn_pool.tile([P, NT, 2, W], BF16, tag="attnT", name=f"attnT{ph}")
            if ph - 1 >= 0 and ph - 1 < NPAIR:
                outp_tiles[ph - 1] = outp_pool.tile([P, NT, P], F32, tag="outp", name=f"outp{ph - 1}")
            for s in range(NT):
                if ph < NPAIR:
                    emit_scores_step(ph, s)
                if ph == 0 and s >= 1:
                    emit_v_step(s - 1)
                    if s == NT - 1:
                        emit_v_step(s)
                if 0 <= ph - 1 < NPAIR and ph != NPAIR:
                    emit_out_step(ph - 1, s)
                if ph == NPAIR and s < 2:
                    emit_out_last(NPAIR - 1, s)
                if ph + 1 < NPAIR:
                    emit_proj_steps(ph + 1, s)
                if ph < NPAIR:
                    if s < 4:
                        if 0 <= ph - 2 < NPAIR:
                            emit_norm_step(ph - 2, 1, s)
                    else:
                        if 0 <= ph - 1 < NPAIR:
                            emit_norm_step(ph - 1, 0, s)
                else:
                    # Epilogue: finish pair NPAIR-2's h1 and all of NPAIR-1.
                    if s < 4:
                        emit_norm_step(NPAIR - 2, 1, s)
                    if 2 <= s < 6:
                        emit_norm_step(NPAIR - 1, 0, s - 2)
                    if s >= 4:
                        emit_norm_step(NPAIR - 1, 1, s - 4)
            if 0 <= ph - 2 < NPAIR:
                emit_out_dma(ph - 2)
            if ph == NPAIR:
                emit_out_dma(NPAIR - 1)

    print(f"[kernel] trace+schedule took {_time.time() - _tb:.1f}s", flush=True)
    _t0 = _time.time()
    nc.compile()
    print(f"[kernel] bacc compile took {_time.time() - _t0:.1f}s", flush=True)
    return nc


_NC_CACHE = None


def _get_nc():
    global _NC_CACHE
    if _NC_CACHE is None:
        _NC_CACHE = build_nc()
    return _NC_CACHE


def _marshal_w(w):
    """(D, H*DK) f32 -> (P, NT, H*DK) bf16 with w[pp, dt, n] = W[128*dt+pp, n]."""
    import ml_dtypes

    w = np.asarray(w, dtype=np.float32).reshape(NT, P, H * DK)
    return np.ascontiguousarray(w.transpose(1, 0, 2)).astype(ml_dtypes.bfloat16)


def _marshal_w_pairmajor(w):
    """(D, H*DK) f32 -> (P, NPAIR, NT, P) bf16 with
    w[pp, pr, dt, c] = W[128*dt+pp, 128*pr+c]."""
    import ml_dtypes

    w = np.asarray(w, dtype=np.float32).reshape(NT, P, NPAIR, P)
    return np.ascontiguousarray(w.transpose(1, 2, 0, 3)).astype(ml_dtypes.bfloat16)


def kernel(inputs, Wq, Wk, Wv, _trace=False):
    import ml_dtypes

    inputs = np.asarray(inputs, dtype=np.float32)
    wq_m = _marshal_w_pairmajor(Wq)
    wk_m = _marshal_w_pairmajor(Wk)
    wv_m = _marshal_w(Wv)
    nc = _get_nc()
    in_maps = []
    for b in range(B):
        xt = inputs[b].T.reshape(NT, P, W)  # [dt, pp, q]
        xt = np.ascontiguousarray(xt.transpose(1, 0, 2)).astype(ml_dtypes.bfloat16)
        in_maps.append({"xt": xt, "wq": wq_m, "wk": wk_m, "wv": wv_m})
    try:
        res = run_bass_kernel_spmd(
            nc, in_maps, core_ids=list(range(B)), trace=_trace
        )
    except Exception:
        # A crashed prior session can leave the device in an unrecoverable
        # state for one execution; a single retry clears it.
        res = run_bass_kernel_spmd(
            nc, in_maps, core_ids=list(range(B)), trace=_trace
        )
    out = np.stack([np.asarray(res.results[b]["out"]) for b in range(B)])
    out = out.reshape(B, W, H, DK).astype(np.float32)
    if _trace:
        return out, res
    return out
